# revision 8
# baseline (speedup 1.0000x reference)
"""Trainium2 Bass kernel for nn_FGN_58007828300291 (Fourier Graph Network).

Math restructure (validated against the reference in fp64 to ~5e-7 rel):
  - tok = outer(xt, emb) is rank-1, and rfft is linear, so the big
    [B, 16384, 128] rfft collapses to a length-16384 rfft of xt per batch:
    Z[b,f,e] = X[b,f] * emb[e].
  - softshrink is always applied to relu outputs here, so
    softshrink(relu(v)) == relu(v - lambda); the elementwise chain becomes
    relu / fused scalar ops only.
  - Only out[:, 0, :] is used downstream, so the irfft + emb10 contraction
    collapses into two precomputed [8, 8193] matrices (Ar, Ai):
       h0[b,e,j] = sum_f Ar[j,f] Sr[b,f,e] + Ai[j,f] Si[b,f,e]
  - S = z + Z splits into the (small) chain part z, pushed through the
    contraction in fp16, and the rank-1 part Z, folded exactly:
       h0 += (Ar @ Xr + Ai @ Xi)[j] * emb[e]   (fp32)
  - forward rfft(16384) = two 128x128 DFT matmul stages + twiddle.

Sharding: pure data parallel, B=16 over 8 cores (2 batches/core). Params are
replicated. Host computes the small weight-derived constant matrices.
"""

import math

import ml_dtypes
import numpy as np

import concourse.bass as bass
import concourse.bacc as bacc
import concourse.tile as tile
from concourse import mybir
from concourse.bass_utils import run_bass_kernel_spmd

F16 = mybir.dt.float16
F32 = mybir.dt.float32
BF16 = mybir.dt.bfloat16
AL = mybir.AluOpType
AF = mybir.ActivationFunctionType

B, L, N = 16, 512, 32
E, EMB = 128, 8
NF = N * L              # 16384
F = NF // 2 + 1         # 8193
K2 = 65                 # f = 128*k2 + k1, k2 in [0, 65)
FP = 128 * K2           # 8320 padded freq count (f' = k1*65 + k2)
LAM = 0.01
NB = 2                  # batches per core
NCORES = 8

# cf (f32 [128, *]) column offsets
O_F1C, O_F1S = 0, 128
O_TWC, O_TWS = 256, 384
O_F2C, O_F2S, O_F2SN = 512, 577, 642
O_A2 = 707              # + k2*16 + ri*8 + j   (65*16 = 1040)
O_W1 = 1747             # + j*64 + c           (512)
O_ID = 2259             # identity 128
CF_COLS = 2387

# chunks over f' (multiples of 128)
CHUNKS = [(i * 1024, 1024) for i in range(8)] + [(8192, 128)]
NTAU = FP // 128        # 65


def _host_constants(inputs):
    """All weight-derived constants, computed once on host in fp64/fp32."""
    emb = np.asarray(inputs["emb"], np.float64)[0]            # [E]
    dg = lambda w: np.diagonal(np.asarray(w, np.float64), axis1=-2, axis2=-1)
    d00, d01 = dg(inputs["w0"][0]), dg(inputs["w0"][1])
    d10, d11 = dg(inputs["w1"][0]), dg(inputs["w1"][1])
    d20, d21 = dg(inputs["w2"][0]), dg(inputs["w2"][1])
    b00, b01 = np.asarray(inputs["b0"], np.float64)
    b10, b11 = np.asarray(inputs["b1"], np.float64)
    b20, b21 = np.asarray(inputs["b2"], np.float64)

    # fused inverse-DFT + emb10 matrices [8, F]
    t = np.arange(L)[:, None].astype(np.float64)
    f = np.arange(F)[None, :].astype(np.float64)
    c = np.full(F, 2.0); c[0] = 1.0; c[-1] = 1.0
    ang = 2.0 * np.pi * t * f / NF
    Cr = c * np.cos(ang) / math.sqrt(NF)
    Ci = -c * np.sin(ang) / math.sqrt(NF)
    emb10 = np.asarray(inputs["emb10"], np.float64)           # [L, 8]
    Ar = emb10.T @ Cr
    Ai = emb10.T @ Ci

    # pad into f' order: f' = k1*65 + k2, f = 128*k2 + k1
    fp_idx = np.arange(FP)
    k1, k2 = fp_idx // K2, fp_idx % K2
    fmap = 128 * k2 + k1
    valid = fmap <= 8192
    Apad = np.zeros((2, 8, FP))
    Apad[0][:, valid] = Ar[:, fmap[valid]]
    Apad[1][:, valid] = Ai[:, fmap[valid]]

    # a16 [128, NTAU*16]: tile tau covers f' in [128*tau, 128*tau+128)
    a16 = np.zeros((128, NTAU * 16), np.float16)
    for tau in range(NTAU):
        for ri in range(2):
            # [p, j] = Apad[ri][j, tau*128 + p]
            a16[:, tau * 16 + ri * 8:tau * 16 + ri * 8 + 8] = \
                Apad[ri][:, tau * 128:(tau + 1) * 128].T.astype(np.float16)

    # cf [128, CF_COLS] f32
    cf = np.zeros((128, CF_COLS), np.float32)
    k = np.arange(128).astype(np.float64)
    kk = np.outer(k, k)
    cf[:, O_F1C:O_F1C + 128] = np.cos(2 * np.pi * kk / 128)
    cf[:, O_F1S:O_F1S + 128] = -np.sin(2 * np.pi * kk / 128)
    cf[:, O_TWC:O_TWC + 128] = np.cos(2 * np.pi * kk / NF)
    cf[:, O_TWS:O_TWS + 128] = -np.sin(2 * np.pi * kk / NF)
    kk2 = np.outer(k, np.arange(K2).astype(np.float64))
    cf[:, O_F2C:O_F2C + K2] = np.cos(2 * np.pi * kk2 / 128) / math.sqrt(NF)
    cf[:, O_F2S:O_F2S + K2] = -np.sin(2 * np.pi * kk2 / 128) / math.sqrt(NF)
    cf[:, O_F2SN:O_F2SN + K2] = np.sin(2 * np.pi * kk2 / 128) / math.sqrt(NF)
    # A2 [p=k1, k2, ri, j] = A[ri][j, 128*k2 + p] (zero beyond 8192)
    Afull = np.zeros((2, 8, 128 * K2))
    Afull[0][:, :F] = Ar; Afull[1][:, :F] = Ai
    for kk2i in range(K2):
        for ri in range(2):
            cf[:, O_A2 + kk2i * 16 + ri * 8:O_A2 + kk2i * 16 + ri * 8 + 8] = \
                Afull[ri][:, kk2i * 128:(kk2i + 1) * 128].T
    # W1j [e, j*64 + c] = fc1_w[c, e*8 + j]
    fc1_w = np.asarray(inputs["fc1_w"], np.float64)           # [64, 1024]
    w1r = fc1_w.reshape(64, 128, 8)                           # [c, e, j]
    for j in range(EMB):
        cf[:, O_W1 + j * 64:O_W1 + (j + 1) * 64] = w1r[:, :, j].T
    cf[:, O_ID:O_ID + 128] = np.eye(128)

    # c3 bf16 [3, 256]: L1 lhsT for o1r (cols 0:128) and o1i (cols 128:256)
    u00, u01 = emb * d00, emb * d01
    c3 = np.zeros((3, 256), np.float32)
    c3[0, :128], c3[1, :128], c3[2, :128] = u00, -u01, b00
    c3[0, 128:], c3[1, 128:], c3[2, 128:] = u01, u00, b01

    # bc16 [128, 12] f16 scalar columns
    bc16 = np.stack([d10, d11, d20, d21,
                     b10, b11, b20, b21,
                     b10 - LAM, b11 - LAM, b20 - LAM, b21 - LAM],
                    axis=1).astype(np.float32)

    w2t = np.asarray(inputs["fc2_w"], np.float32).T.copy()    # [64, 256]
    fc3_w = np.asarray(inputs["fc3_w"], np.float64)           # [96, 256]
    w3t = np.zeros((128, 192), np.float32)
    w3t[:, 0:96] = fc3_w[:, 0:128].T
    w3t[:, 96:192] = fc3_w[:, 128:256].T
    fcb = np.zeros((128, 4), np.float32)
    fcb[0:64, 0] = np.asarray(inputs["fc1_b"], np.float32)
    fcb[:, 1] = np.asarray(inputs["fc2_b"], np.float32)[0:128]
    fcb[:, 2] = np.asarray(inputs["fc2_b"], np.float32)[128:256]
    fcb[0:96, 3] = np.asarray(inputs["fc3_b"], np.float32)
    embr = np.asarray(inputs["emb"], np.float32)              # [1, 128]

    return {
        "cf": cf, "c3": c3.astype(ml_dtypes.bfloat16), "a16": a16, "bc16": bc16,
        "w2t": w2t, "w3t": w3t, "fcb": fcb, "embr": embr,
    }


DEBUG = False


def _build_nc():
    nc = bacc.Bacc(None)
    x2_d = nc.dram_tensor("x2", [NB, 128, 128], F32, kind="ExternalInput")
    cf_d = nc.dram_tensor("cf", [128, CF_COLS], F32, kind="ExternalInput")
    c3_d = nc.dram_tensor("c3", [3, 256], BF16, kind="ExternalInput")
    a16_d = nc.dram_tensor("a16", [128, NTAU * 16], F16, kind="ExternalInput")
    bc16_d = nc.dram_tensor("bc16", [128, 12], F32, kind="ExternalInput")
    w2t_d = nc.dram_tensor("w2t", [64, 256], F32, kind="ExternalInput")
    w3t_d = nc.dram_tensor("w3t", [128, 192], F32, kind="ExternalInput")
    fcb_d = nc.dram_tensor("fcb", [128, 4], F32, kind="ExternalInput")
    embr_d = nc.dram_tensor("embr", [1, 128], F32, kind="ExternalInput")
    out_d = nc.dram_tensor("out", [96, NB], F32, kind="ExternalOutput")
    if DEBUG:
        dbg_x = nc.dram_tensor("dbg_x", [128, K2, 2, NB], F32, kind="ExternalOutput")
        dbg_xs = nc.dram_tensor("dbg_xs", [3, NB, FP], BF16, kind="ExternalOutput")
        dbg_zr = nc.dram_tensor("dbg_zr", [128, NB, FP], F16, kind="ExternalOutput")
        dbg_zi = nc.dram_tensor("dbg_zi", [128, NB, FP], F16, kind="ExternalOutput")
        dbg_h0 = nc.dram_tensor("dbg_h0", [8, NB * 128], F32, kind="ExternalOutput")
        dbg_v = nc.dram_tensor("dbg_v", [8, NB * 2], F32, kind="ExternalOutput")
        dbg_hv = nc.dram_tensor("dbg_hv", [128, NB, 8], F32, kind="ExternalOutput")
        dbg_o1 = nc.dram_tensor("dbg_o1", [128, 2, 1024], F16, kind="ExternalOutput")

    with tile.TileContext(nc) as tc:
        with (
            tc.tile_pool(name="consts", bufs=1) as C,
            tc.tile_pool(name="chain", bufs=3) as CH,
            tc.tile_pool(name="zacc", bufs=1) as Z,
            tc.tile_pool(name="zt", bufs=2) as ZT,
            tc.tile_pool(name="psA", bufs=1, space="PSUM") as PSA,
            tc.tile_pool(name="psacc", bufs=1, space="PSUM") as PACC,
            tc.tile_pool(name="pssm", bufs=2, space="PSUM") as PSM,
        ):
            # ---- load constants ----
            cf = C.tile([128, CF_COLS], F32)
            nc.gpsimd.dma_start(out=cf, in_=cf_d[:, :])
            c3 = C.tile([3, 256], BF16)
            nc.gpsimd.dma_start(out=c3, in_=c3_d[:, :])
            a16 = C.tile([128, NTAU * 16], F16)
            nc.gpsimd.dma_start(out=a16, in_=a16_d[:, :])
            bc = C.tile([128, 12], F32)
            nc.gpsimd.dma_start(out=bc, in_=bc16_d[:, :])
            w2t = C.tile([64, 256], F32)
            nc.gpsimd.dma_start(out=w2t, in_=w2t_d[:, :])
            w3t = C.tile([128, 192], F32)
            nc.gpsimd.dma_start(out=w3t, in_=w3t_d[:, :])
            fcb = C.tile([128, 4], F32)
            nc.gpsimd.dma_start(out=fcb, in_=fcb_d[:, :])
            emb8 = C.tile([8, 128], F32)
            embr_ap = embr_d[:, :]
            nc.gpsimd.dma_start(
                out=emb8,
                in_=bass.AP(tensor=embr_ap.tensor, offset=embr_ap.offset,
                            ap=[[0, 8], [1, 128]]))

            xstage = C.tile([3, NB, FP], BF16)
            nc.vector.memset(xstage[:, :, :], 1.0)
            x_sb = C.tile([128, K2, 2, NB], F32)       # [k1, k2, ri, b]
            xbf = C.tile([128, K2, 2, NB], BF16)
            zacc_r = Z.tile([128, NB, FP], F16)
            zacc_i = Z.tile([128, NB, FP], F16)

            h0ps = PACC.tile([8, NB * 128], F32, tag="h0")
            vps = PACC.tile([8, NB * 2], F32, tag="v")

            # ---- forward FFT per batch ----
            for b in range(NB):
                x2 = CH.tile([128, 128], F32, tag="x2")
                nc.sync.dma_start(out=x2, in_=x2_d[b, :, :])
                ps1 = PSM.tile([128, 2, 128], F32, tag="sm")
                nc.tensor.matmul(ps1[:, 0, :], x2, cf[:, O_F1C:O_F1C + 128],
                                 start=True, stop=True)
                nc.tensor.matmul(ps1[:, 1, :], x2, cf[:, O_F1S:O_F1S + 128],
                                 start=True, stop=True)
                # twiddle (complex mult by W[t2, k1])
                ap_r = CH.tile([128, 128], F32, tag="apr")
                ap_i = CH.tile([128, 128], F32, tag="api")
                tw1 = CH.tile([128, 128], F32, tag="tw1")
                tw2 = CH.tile([128, 128], F32, tag="tw2")
                nc.vector.tensor_mul(tw1, ps1[:, 0, :], cf[:, O_TWC:O_TWC + 128])
                nc.vector.tensor_mul(tw2, ps1[:, 1, :], cf[:, O_TWS:O_TWS + 128])
                nc.vector.tensor_sub(ap_r, tw1, tw2)
                nc.vector.tensor_mul(tw1, ps1[:, 0, :], cf[:, O_TWS:O_TWS + 128])
                nc.vector.tensor_mul(tw2, ps1[:, 1, :], cf[:, O_TWC:O_TWC + 128])
                nc.vector.tensor_add(ap_i, tw1, tw2)
                ps2 = PSM.tile([128, 2, 128], F32, tag="sm")
                nc.tensor.matmul(ps2[:, 0, 0:K2], ap_r, cf[:, O_F2C:O_F2C + K2],
                                 start=True, stop=False)
                nc.tensor.matmul(ps2[:, 0, 0:K2], ap_i, cf[:, O_F2SN:O_F2SN + K2],
                                 start=False, stop=True)
                nc.tensor.matmul(ps2[:, 1, 0:K2], ap_r, cf[:, O_F2S:O_F2S + K2],
                                 start=True, stop=False)
                nc.tensor.matmul(ps2[:, 1, 0:K2], ap_i, cf[:, O_F2C:O_F2C + K2],
                                 start=False, stop=True)
                for ri in range(2):
                    nc.vector.tensor_copy(x_sb[:, :, ri, b], ps2[:, ri, 0:K2])
                    nc.scalar.copy(xbf[:, :, ri, b], ps2[:, ri, 0:K2])
                    # row-major flatten [128, 65] -> f'-ordered row [1, 8320]
                    nc.sync.dma_start(out=xstage[ri:ri + 1, b, :],
                                      in_=xbf[:, :, ri, b])

            if DEBUG:
                nc.sync.dma_start(out=dbg_x[:, :, :, :], in_=x_sb)
                nc.sync.dma_start(out=dbg_xs[:, :, :], in_=xstage)

            # ---- rank-1 fold: v[j, (b, ri-summed)] over 130 tiny matmuls ----
            nmm = 2 * K2
            mi = 0
            for k2i in range(K2):
                for ri in range(2):
                    a2 = cf[:, O_A2 + k2i * 16 + ri * 8:O_A2 + k2i * 16 + ri * 8 + 8]
                    nc.tensor.matmul(vps[:, 0:NB], a2, x_sb[:, k2i, ri, :],
                                     start=(mi == 0), stop=(mi == nmm - 1))
                    mi += 1

            # ---- elementwise chain + transpose + A-contraction ----
            nacc = 2 * 2 * NTAU      # accumulating matmuls into h0ps
            acc_i = 0
            for ci, (c0, cw) in enumerate(CHUNKS):
                nt = cw // 128
                zt_r = ZT.tile([128, 8, NB, 128], F16, tag="ztr")
                zt_i = ZT.tile([128, 8, NB, 128], F16, tag="zti")
                for b in range(NB):
                    psA = PSA.tile([128, 2, 1024], F32, tag="psA")
                    for h0_ in range(0, cw, 512):
                        hw_ = min(512, cw - h0_)
                        nc.tensor.matmul(psA[:, 0, h0_:h0_ + hw_], c3[:, 0:128],
                                         xstage[:, b, c0 + h0_:c0 + h0_ + hw_],
                                         start=True, stop=True)
                        nc.tensor.matmul(psA[:, 1, h0_:h0_ + hw_], c3[:, 128:256],
                                         xstage[:, b, c0 + h0_:c0 + h0_ + hw_],
                                         start=True, stop=True)
                    o1r = CH.tile([128, 1024], F16, tag="o1r")
                    o1i = CH.tile([128, 1024], F16, tag="o1i")
                    nc.scalar.activation(o1r[:, 0:cw], psA[:, 0, 0:cw], AF.Relu)
                    nc.scalar.activation(o1i[:, 0:cw], psA[:, 1, 0:cw], AF.Relu)
                    nc.vector.tensor_scalar(out=zacc_r[:, b, c0:c0 + cw],
                                            in0=psA[:, 0, 0:cw],
                                            scalar1=LAM, scalar2=0.0,
                                            op0=AL.subtract, op1=AL.max)
                    nc.vector.tensor_scalar(out=zacc_i[:, b, c0:c0 + cw],
                                            in0=psA[:, 1, 0:cw],
                                            scalar1=LAM, scalar2=0.0,
                                            op0=AL.subtract, op1=AL.max)

                    if DEBUG and ci == 0 and b == 0:
                        nc.sync.dma_start(out=dbg_o1[:, 0, :], in_=o1r)
                        nc.sync.dma_start(out=dbg_o1[:, 1, :], in_=o1i)
                    t1 = CH.tile([128, 1024], F16, tag="t1")
                    t2 = CH.tile([128, 1024], F16, tag="t2")
                    t3r = CH.tile([128, 1024], F16, tag="t3r")
                    t3i = CH.tile([128, 1024], F16, tag="t3i")
                    ctb = CH.tile([128, 1024], F16, tag="ctb")

                    def layer(oa, ob, dcol0, dcol1, bcol0, bcol1, blam0, blam1,
                              extract_oi, tag):
                        """one fourierGC layer: returns (o_new_r, o_new_i?)"""
                        # t3r = oa*d0 - ob*d1
                        nc.vector.tensor_scalar_mul(t1[:, 0:cw], oa[:, 0:cw],
                                                    bc[:, dcol0:dcol0 + 1])
                        nc.vector.tensor_scalar_mul(t2[:, 0:cw], ob[:, 0:cw],
                                                    bc[:, dcol1:dcol1 + 1])
                        nc.vector.tensor_sub(t3r[:, 0:cw], t1[:, 0:cw], t2[:, 0:cw])
                        onr = CH.tile([128, 1024], F16, tag=tag + "r")
                        nc.gpsimd.tensor_scalar(out=onr[:, 0:cw], in0=t3r[:, 0:cw],
                                                scalar1=bc[:, bcol0:bcol0 + 1],
                                                scalar2=0.0,
                                                op0=AL.add, op1=AL.max)
                        nc.vector.tensor_scalar(out=ctb[:, 0:cw], in0=t3r[:, 0:cw],
                                                scalar1=bc[:, blam0:blam0 + 1],
                                                scalar2=0.0,
                                                op0=AL.add, op1=AL.max)
                        nc.vector.tensor_add(zacc_r[:, b, c0:c0 + cw],
                                             zacc_r[:, b, c0:c0 + cw],
                                             ctb[:, 0:cw])
                        # t3i = ob*d0 + onr*d1
                        nc.vector.tensor_scalar_mul(t1[:, 0:cw], ob[:, 0:cw],
                                                    bc[:, dcol0:dcol0 + 1])
                        nc.vector.tensor_scalar_mul(t2[:, 0:cw], onr[:, 0:cw],
                                                    bc[:, dcol1:dcol1 + 1])
                        nc.vector.tensor_add(t3i[:, 0:cw], t1[:, 0:cw], t2[:, 0:cw])
                        oni = None
                        if extract_oi:
                            oni = CH.tile([128, 1024], F16, tag=tag + "i")
                            nc.gpsimd.tensor_scalar(out=oni[:, 0:cw],
                                                    in0=t3i[:, 0:cw],
                                                    scalar1=bc[:, bcol1:bcol1 + 1],
                                                    scalar2=0.0,
                                                    op0=AL.add, op1=AL.max)
                        nc.vector.tensor_scalar(out=ctb[:, 0:cw], in0=t3i[:, 0:cw],
                                                scalar1=bc[:, blam1:blam1 + 1],
                                                scalar2=0.0,
                                                op0=AL.add, op1=AL.max)
                        nc.vector.tensor_add(zacc_i[:, b, c0:c0 + cw],
                                             zacc_i[:, b, c0:c0 + cw],
                                             ctb[:, 0:cw])
                        return onr, oni

                    o2r, o2i = layer(o1r, o1i, 0, 1, 4, 5, 8, 9, True, "o2")
                    layer(o2r, o2i, 2, 3, 6, 7, 10, 11, False, "o3")

                    nc.sync.dma_start_transpose(out=zt_r[:, 0:nt, b, :],
                                                in_=zacc_r[:, b, c0:c0 + cw])
                    nc.sync.dma_start_transpose(out=zt_i[:, 0:nt, b, :],
                                                in_=zacc_i[:, b, c0:c0 + cw])

                for t in range(nt):
                    tau = ci * 8 + t
                    for ri, ztt in ((0, zt_r), (1, zt_i)):
                        nc.tensor.matmul(
                            h0ps[:, :],
                            a16[:, tau * 16 + ri * 8:tau * 16 + ri * 8 + 8],
                            ztt[:, t, :, :],
                            start=(acc_i == 0), stop=(acc_i == nacc - 1))
                        acc_i += 1

            # ---- h0 completion + FC head ----
            v_sb = CH.tile([8, NB * 2], F32, tag="vsb")
            nc.vector.tensor_copy(v_sb, vps[:, :])
            h1in = CH.tile([8, NB, 128], F32, tag="h1in")
            tmp8 = CH.tile([8, 128], F32, tag="tmp8")
            for b in range(NB):
                nc.vector.tensor_scalar_mul(tmp8, emb8, v_sb[:, b:b + 1])
                nc.vector.tensor_add(h1in[:, b, :], tmp8,
                                     h0ps[:, b * 128:(b + 1) * 128])
            pst = PSM.tile([128, 2, 128], F32, tag="sm")
            hv = CH.tile([128, NB, 8], F32, tag="hv")
            for b in range(NB):
                nc.tensor.transpose(pst[:, b, 0:8], h1in[:, b, :],
                                    cf[0:8, O_ID:O_ID + 8])
                nc.vector.tensor_copy(hv[:, b, :], pst[:, b, 0:8])
            psf1 = PSM.tile([128, 2, 128], F32, tag="sm")
            for j in range(EMB):
                nc.tensor.matmul(psf1[0:64, 0, 0:NB],
                                 cf[:, O_W1 + j * 64:O_W1 + (j + 1) * 64],
                                 hv[:, :, j], start=(j == 0), stop=(j == EMB - 1))
            h1 = CH.tile([64, NB], F32, tag="h1")
            nc.scalar.activation(h1, psf1[0:64, 0, 0:NB], AF.Lrelu,
                                 bias=fcb[0:64, 0:1], scale=1.0, alpha=0.01)
            psf2 = PSM.tile([128, 2, 128], F32, tag="sm")
            h2 = CH.tile([128, 2, NB], F32, tag="h2")
            for h in range(2):
                nc.tensor.matmul(psf2[:, h, 0:NB], w2t[:, h * 128:(h + 1) * 128],
                                 h1, start=True, stop=True)
                nc.scalar.activation(h2[:, h, :], psf2[:, h, 0:NB], AF.Lrelu,
                                     bias=fcb[:, 1 + h:2 + h], scale=1.0,
                                     alpha=0.01)
            psf3 = PSM.tile([128, 2, 128], F32, tag="sm")
            for h in range(2):
                nc.tensor.matmul(psf3[0:96, 0, 0:NB],
                                 w3t[:, h * 96:(h + 1) * 96],
                                 h2[:, h, :], start=(h == 0), stop=(h == 1))
            if DEBUG:
                nc.sync.dma_start(out=dbg_zr[:, :, :], in_=zacc_r)
                nc.sync.dma_start(out=dbg_zi[:, :, :], in_=zacc_i)
                h0_sb = CH.tile([8, NB * 128], F32, tag="h0sb")
                nc.vector.tensor_copy(h0_sb, h0ps[:, :])
                nc.sync.dma_start(out=dbg_h0[:, :], in_=h0_sb)
                nc.sync.dma_start(out=dbg_v[:, :], in_=v_sb)
                nc.sync.dma_start(out=dbg_hv[:, :, :], in_=hv)
            out_sb = CH.tile([96, NB], F32, tag="outsb")
            nc.vector.tensor_scalar_add(out_sb, psf3[0:96, 0, 0:NB],
                                        fcb[0:96, 3:4])
            nc.sync.dma_start(out=out_d[:, :], in_=out_sb)

    nc.finalize()
    return nc


_NC_CACHE = None


def kernel(**inputs) -> np.ndarray:
    global _NC_CACHE
    if _NC_CACHE is None:
        _NC_CACHE = _build_nc()
    nc = _NC_CACHE

    consts = _host_constants(inputs)
    x = np.asarray(inputs["x"], np.float32)                   # [16, 512, 32]
    # xt[b] = x[b].T.flatten(); x2 = xt.reshape(128, 128)
    x2 = np.ascontiguousarray(x.transpose(0, 2, 1)).reshape(B, 128, 128)

    in_maps = []
    for c in range(NCORES):
        m = {"x2": x2[c * NB:(c + 1) * NB]}
        m.update(consts)
        in_maps.append(m)
    res = run_bass_kernel_spmd(nc, in_maps, core_ids=list(range(NCORES)))
    out = np.concatenate([r["out"].T for r in res.results], axis=0)
    return out.astype(np.float32)


# revision 9
# speedup vs baseline: 4.8771x; 4.8771x over previous
"""Trainium2 Bass kernel for nn_FGN_58007828300291 (Fourier Graph Network).

Math restructure (validated against the reference in fp64 to ~5e-7 rel):
  - tok = outer(xt, emb) is rank-1, and rfft is linear, so the big
    [B, 16384, 128] rfft collapses to a length-16384 rfft of xt per batch:
    Z[b,f,e] = X[b,f] * emb[e].
  - softshrink is always applied to relu outputs here, so
    softshrink(relu(v)) == relu(v - lambda); the elementwise chain becomes
    relu / fused scalar ops only.
  - Only out[:, 0, :] is used downstream, so the irfft + emb10 contraction
    collapses into two precomputed [8, 8193] matrices (Ar, Ai):
       h0[b,e,j] = sum_f Ar[j,f] Sr[b,f,e] + Ai[j,f] Si[b,f,e]
  - S = z + Z splits into the (small) chain part z, pushed through the
    contraction in fp16, and the rank-1 part Z, folded exactly:
       h0 += (Ar @ Xr + Ai @ Xi)[j] * emb[e]   (fp32)
  - forward rfft(16384) = two 128x128 DFT matmul stages + twiddle.

Sharding: pure data parallel, B=16 over 8 cores (2 batches/core). Params are
replicated. Host computes the small weight-derived constant matrices.
"""

import math

import ml_dtypes
import numpy as np

import concourse.bass as bass
import concourse.bacc as bacc
import concourse.tile as tile
from concourse import mybir
from concourse.bass_utils import run_bass_kernel_spmd

F16 = mybir.dt.float16
F32 = mybir.dt.float32
BF16 = mybir.dt.bfloat16
AL = mybir.AluOpType
AF = mybir.ActivationFunctionType

B, L, N = 16, 512, 32
E, EMB = 128, 8
NF = N * L              # 16384
F = NF // 2 + 1         # 8193
K2 = 65                 # f = 128*k2 + k1, k2 in [0, 65)
FP = 128 * K2           # 8320 padded freq count (f' = k1*65 + k2)
LAM = 0.01
NB = 2                  # batches per core
NCORES = 8

# cf (f32 [128, *]) column offsets
O_F1C, O_F1S = 0, 128
O_TWC, O_TWS = 256, 384
O_F2C, O_F2S, O_F2SN = 512, 577, 642
O_A2 = 707              # + k2*16 + ri*8 + j   (65*16 = 1040)
O_W1 = 1747             # + j*64 + c           (512)
O_ID = 2259             # identity 128
CF_COLS = 2387

# chunks over f' (multiples of 128)
CHUNKS = [(i * 1024, 1024) for i in range(8)] + [(8192, 128)]
NTAU = FP // 128        # 65


def _host_constants(inputs):
    """All weight-derived constants, computed once on host in fp64/fp32."""
    emb = np.asarray(inputs["emb"], np.float64)[0]            # [E]
    dg = lambda w: np.diagonal(np.asarray(w, np.float64), axis1=-2, axis2=-1)
    d00, d01 = dg(inputs["w0"][0]), dg(inputs["w0"][1])
    d10, d11 = dg(inputs["w1"][0]), dg(inputs["w1"][1])
    d20, d21 = dg(inputs["w2"][0]), dg(inputs["w2"][1])
    b00, b01 = np.asarray(inputs["b0"], np.float64)
    b10, b11 = np.asarray(inputs["b1"], np.float64)
    b20, b21 = np.asarray(inputs["b2"], np.float64)

    # fused inverse-DFT + emb10 matrices [8, F]
    t = np.arange(L)[:, None].astype(np.float64)
    f = np.arange(F)[None, :].astype(np.float64)
    c = np.full(F, 2.0); c[0] = 1.0; c[-1] = 1.0
    ang = 2.0 * np.pi * t * f / NF
    Cr = c * np.cos(ang) / math.sqrt(NF)
    Ci = -c * np.sin(ang) / math.sqrt(NF)
    emb10 = np.asarray(inputs["emb10"], np.float64)           # [L, 8]
    Ar = emb10.T @ Cr
    Ai = emb10.T @ Ci

    # pad into f' order: f' = k1*65 + k2, f = 128*k2 + k1
    fp_idx = np.arange(FP)
    k1, k2 = fp_idx // K2, fp_idx % K2
    fmap = 128 * k2 + k1
    valid = fmap <= 8192
    Apad = np.zeros((2, 8, FP))
    Apad[0][:, valid] = Ar[:, fmap[valid]]
    Apad[1][:, valid] = Ai[:, fmap[valid]]

    # a16 [128, NTAU*16]: tile tau covers f' in [128*tau, 128*tau+128)
    a16 = np.zeros((128, NTAU * 16), np.float16)
    for tau in range(NTAU):
        for ri in range(2):
            # [p, j] = Apad[ri][j, tau*128 + p]
            a16[:, tau * 16 + ri * 8:tau * 16 + ri * 8 + 8] = \
                Apad[ri][:, tau * 128:(tau + 1) * 128].T.astype(np.float16)

    # cf [128, CF_COLS] f32
    cf = np.zeros((128, CF_COLS), np.float32)
    k = np.arange(128).astype(np.float64)
    kk = np.outer(k, k)
    cf[:, O_F1C:O_F1C + 128] = np.cos(2 * np.pi * kk / 128)
    cf[:, O_F1S:O_F1S + 128] = -np.sin(2 * np.pi * kk / 128)
    cf[:, O_TWC:O_TWC + 128] = np.cos(2 * np.pi * kk / NF)
    cf[:, O_TWS:O_TWS + 128] = -np.sin(2 * np.pi * kk / NF)
    kk2 = np.outer(k, np.arange(K2).astype(np.float64))
    cf[:, O_F2C:O_F2C + K2] = np.cos(2 * np.pi * kk2 / 128) / math.sqrt(NF)
    cf[:, O_F2S:O_F2S + K2] = -np.sin(2 * np.pi * kk2 / 128) / math.sqrt(NF)
    cf[:, O_F2SN:O_F2SN + K2] = np.sin(2 * np.pi * kk2 / 128) / math.sqrt(NF)
    # A2 [p=k1, k2, ri, j] = A[ri][j, 128*k2 + p] (zero beyond 8192)
    Afull = np.zeros((2, 8, 128 * K2))
    Afull[0][:, :F] = Ar; Afull[1][:, :F] = Ai
    for kk2i in range(K2):
        for ri in range(2):
            cf[:, O_A2 + kk2i * 16 + ri * 8:O_A2 + kk2i * 16 + ri * 8 + 8] = \
                Afull[ri][:, kk2i * 128:(kk2i + 1) * 128].T
    # W1j [e, j*64 + c] = fc1_w[c, e*8 + j]
    fc1_w = np.asarray(inputs["fc1_w"], np.float64)           # [64, 1024]
    w1r = fc1_w.reshape(64, 128, 8)                           # [c, e, j]
    for j in range(EMB):
        cf[:, O_W1 + j * 64:O_W1 + (j + 1) * 64] = w1r[:, :, j].T
    cf[:, O_ID:O_ID + 128] = np.eye(128)

    # c3 bf16 [3, 256]: L1 lhsT for o1r (cols 0:128) and o1i (cols 128:256)
    u00, u01 = emb * d00, emb * d01
    c3 = np.zeros((2, 256), np.float32)
    c3[0, :128], c3[1, :128] = u00, -u01
    c3[0, 128:], c3[1, 128:] = u01, u00

    # bc16 [128, 12] f16 scalar columns
    bc16 = np.stack([d10, d11, d20, d21,
                     b10, b11, b20, b21,
                     b10 - LAM, b11 - LAM, b20 - LAM, b21 - LAM,
                     b00 - LAM, b01 - LAM, b00, b01],
                    axis=1).astype(np.float32)

    w2t = np.asarray(inputs["fc2_w"], np.float32).T.copy()    # [64, 256]
    fc3_w = np.asarray(inputs["fc3_w"], np.float64)           # [96, 256]
    w3t = np.zeros((128, 192), np.float32)
    w3t[:, 0:96] = fc3_w[:, 0:128].T
    w3t[:, 96:192] = fc3_w[:, 128:256].T
    fcb = np.zeros((128, 4), np.float32)
    fcb[0:64, 0] = np.asarray(inputs["fc1_b"], np.float32)
    fcb[:, 1] = np.asarray(inputs["fc2_b"], np.float32)[0:128]
    fcb[:, 2] = np.asarray(inputs["fc2_b"], np.float32)[128:256]
    fcb[0:96, 3] = np.asarray(inputs["fc3_b"], np.float32)
    embr = np.asarray(inputs["emb"], np.float32)              # [1, 128]

    return {
        "cf": cf, "c3": c3.astype(ml_dtypes.bfloat16), "a16": a16, "bc16": bc16,
        "w2t": w2t, "w3t": w3t, "fcb": fcb, "embr": embr,
    }


DEBUG = False


def _build_nc():
    nc = bacc.Bacc(None)
    x2_d = nc.dram_tensor("x2", [NB, 128, 128], F32, kind="ExternalInput")
    cf_d = nc.dram_tensor("cf", [128, CF_COLS], F32, kind="ExternalInput")
    c3_d = nc.dram_tensor("c3", [2, 256], BF16, kind="ExternalInput")
    a16_d = nc.dram_tensor("a16", [128, NTAU * 16], F16, kind="ExternalInput")
    bc16_d = nc.dram_tensor("bc16", [128, 16], F32, kind="ExternalInput")
    w2t_d = nc.dram_tensor("w2t", [64, 256], F32, kind="ExternalInput")
    w3t_d = nc.dram_tensor("w3t", [128, 192], F32, kind="ExternalInput")
    fcb_d = nc.dram_tensor("fcb", [128, 4], F32, kind="ExternalInput")
    embr_d = nc.dram_tensor("embr", [1, 128], F32, kind="ExternalInput")
    out_d = nc.dram_tensor("out", [96, NB], F32, kind="ExternalOutput")
    if DEBUG:
        dbg_x = nc.dram_tensor("dbg_x", [128, K2, 2, NB], F32, kind="ExternalOutput")
        dbg_xs = nc.dram_tensor("dbg_xs", [2, NB, FP], BF16, kind="ExternalOutput")
        dbg_zr = nc.dram_tensor("dbg_zr", [128, NB, FP], F16, kind="ExternalOutput")
        dbg_zi = nc.dram_tensor("dbg_zi", [128, NB, FP], F16, kind="ExternalOutput")
        dbg_h0 = nc.dram_tensor("dbg_h0", [8, NB * 128], F32, kind="ExternalOutput")
        dbg_v = nc.dram_tensor("dbg_v", [8, NB * 2], F32, kind="ExternalOutput")
        dbg_hv = nc.dram_tensor("dbg_hv", [128, NB, 8], F32, kind="ExternalOutput")
        dbg_o1 = nc.dram_tensor("dbg_o1", [128, 2, 1024], F16, kind="ExternalOutput")

    with tile.TileContext(nc) as tc:
        with (
            tc.tile_pool(name="consts", bufs=1) as C,
            tc.tile_pool(name="chain", bufs=3) as CH,
            tc.tile_pool(name="zacc", bufs=1) as Z,
            tc.tile_pool(name="zt", bufs=2) as ZT,
            tc.tile_pool(name="psA", bufs=1, space="PSUM") as PSA,
            tc.tile_pool(name="psacc", bufs=1, space="PSUM") as PACC,
            tc.tile_pool(name="pssm", bufs=2, space="PSUM") as PSM,
        ):
            # ---- load constants ----
            cf = C.tile([128, CF_COLS], F32)
            nc.gpsimd.dma_start(out=cf, in_=cf_d[:, :])
            c3 = C.tile([2, 256], BF16)
            nc.gpsimd.dma_start(out=c3, in_=c3_d[:, :])
            a16 = C.tile([128, NTAU * 16], F16)
            nc.gpsimd.dma_start(out=a16, in_=a16_d[:, :])
            bc = C.tile([128, 16], F32)
            nc.gpsimd.dma_start(out=bc, in_=bc16_d[:, :])
            w2t = C.tile([64, 256], F32)
            nc.gpsimd.dma_start(out=w2t, in_=w2t_d[:, :])
            w3t = C.tile([128, 192], F32)
            nc.gpsimd.dma_start(out=w3t, in_=w3t_d[:, :])
            fcb = C.tile([128, 4], F32)
            nc.gpsimd.dma_start(out=fcb, in_=fcb_d[:, :])
            emb8 = C.tile([8, 128], F32)
            embr_ap = embr_d[:, :]
            nc.gpsimd.dma_start(
                out=emb8,
                in_=bass.AP(tensor=embr_ap.tensor, offset=embr_ap.offset,
                            ap=[[0, 8], [1, 128]]))

            xstage = C.tile([2, NB, FP], BF16)
            x_sb = C.tile([128, K2, 2, NB], F32)       # [k1, k2, ri, b]
            xbf = C.tile([128, 2, NB, K2], BF16)
            zacc_r = Z.tile([128, NB, FP], F16)
            zacc_i = Z.tile([128, NB, FP], F16)

            h0ps = PACC.tile([8, NB * 128], F32, tag="h0")
            vps = PACC.tile([8, NB * 2], F32, tag="v")

            # ---- forward FFT per batch ----
            for b in range(NB):
                x2 = CH.tile([128, 128], F32, tag="x2")
                nc.sync.dma_start(out=x2, in_=x2_d[b, :, :])
                ps1 = PSM.tile([128, 2, 128], F32, tag="sm")
                nc.tensor.matmul(ps1[:, 0, :], x2, cf[:, O_F1C:O_F1C + 128],
                                 start=True, stop=True)
                nc.tensor.matmul(ps1[:, 1, :], x2, cf[:, O_F1S:O_F1S + 128],
                                 start=True, stop=True)
                # twiddle (complex mult by W[t2, k1])
                ap_r = CH.tile([128, 128], F32, tag="apr")
                ap_i = CH.tile([128, 128], F32, tag="api")
                tw1 = CH.tile([128, 128], F32, tag="tw1")
                tw2 = CH.tile([128, 128], F32, tag="tw2")
                nc.vector.tensor_mul(tw1, ps1[:, 0, :], cf[:, O_TWC:O_TWC + 128])
                nc.vector.tensor_mul(tw2, ps1[:, 1, :], cf[:, O_TWS:O_TWS + 128])
                nc.vector.tensor_sub(ap_r, tw1, tw2)
                nc.vector.tensor_mul(tw1, ps1[:, 0, :], cf[:, O_TWS:O_TWS + 128])
                nc.vector.tensor_mul(tw2, ps1[:, 1, :], cf[:, O_TWC:O_TWC + 128])
                nc.vector.tensor_add(ap_i, tw1, tw2)
                ps2 = PSM.tile([128, 2, 128], F32, tag="sm")
                nc.tensor.matmul(ps2[:, 0, 0:K2], ap_r, cf[:, O_F2C:O_F2C + K2],
                                 start=True, stop=False)
                nc.tensor.matmul(ps2[:, 0, 0:K2], ap_i, cf[:, O_F2SN:O_F2SN + K2],
                                 start=False, stop=True)
                nc.tensor.matmul(ps2[:, 1, 0:K2], ap_r, cf[:, O_F2S:O_F2S + K2],
                                 start=True, stop=False)
                nc.tensor.matmul(ps2[:, 1, 0:K2], ap_i, cf[:, O_F2C:O_F2C + K2],
                                 start=False, stop=True)
                for ri in range(2):
                    nc.vector.tensor_copy(x_sb[:, :, ri, b], ps2[:, ri, 0:K2])
                    nc.scalar.copy(xbf[:, ri, b, :], ps2[:, ri, 0:K2])
                    # row-major flatten [128, 65] -> f'-ordered row [1, 8320]
                    nc.sync.dma_start(out=xstage[ri:ri + 1, b, :],
                                      in_=xbf[:, ri, b, :])

            if DEBUG:
                nc.sync.dma_start(out=dbg_x[:, :, :, :], in_=x_sb)
                nc.sync.dma_start(out=dbg_xs[:, :, :], in_=xstage)

            # ---- rank-1 fold: v[j, (b, ri-summed)] over 130 tiny matmuls ----
            nmm = 2 * K2
            mi = 0
            for k2i in range(K2):
                for ri in range(2):
                    a2 = cf[:, O_A2 + k2i * 16 + ri * 8:O_A2 + k2i * 16 + ri * 8 + 8]
                    nc.tensor.matmul(vps[:, 0:NB], a2, x_sb[:, k2i, ri, :],
                                     start=(mi == 0), stop=(mi == nmm - 1))
                    mi += 1

            # ---- elementwise chain + transpose + A-contraction ----
            nacc = 2 * 2 * NTAU      # accumulating matmuls into h0ps
            acc_i = 0
            for ci, (c0, cw) in enumerate(CHUNKS):
                nt = cw // 128
                zt_r = ZT.tile([128, 8, NB, 128], F16, tag="ztr")
                zt_i = ZT.tile([128, 8, NB, 128], F16, tag="zti")
                for b in range(NB):
                    psA = PSA.tile([128, 2, 1024], F32, tag="psA")
                    for h0_ in range(0, cw, 512):
                        hw_ = min(512, cw - h0_)
                        nc.tensor.matmul(psA[:, 0, h0_:h0_ + hw_], c3[:, 0:128],
                                         xstage[:, b, c0 + h0_:c0 + h0_ + hw_],
                                         start=True, stop=True)
                        nc.tensor.matmul(psA[:, 1, h0_:h0_ + hw_], c3[:, 128:256],
                                         xstage[:, b, c0 + h0_:c0 + h0_ + hw_],
                                         start=True, stop=True)
                    o1r = CH.tile([128, 1024], F16, tag="o1r")
                    o1i = CH.tile([128, 1024], F16, tag="o1i")
                    nc.scalar.activation(o1r[:, 0:cw], psA[:, 0, 0:cw], AF.Relu,
                                         bias=bc[:, 14:15])
                    nc.scalar.activation(o1i[:, 0:cw], psA[:, 1, 0:cw], AF.Relu,
                                         bias=bc[:, 15:16])
                    nc.vector.tensor_scalar(out=zacc_r[:, b, c0:c0 + cw],
                                            in0=psA[:, 0, 0:cw],
                                            scalar1=bc[:, 12:13], scalar2=0.0,
                                            op0=AL.add, op1=AL.max)
                    nc.vector.tensor_scalar(out=zacc_i[:, b, c0:c0 + cw],
                                            in0=psA[:, 1, 0:cw],
                                            scalar1=bc[:, 13:14], scalar2=0.0,
                                            op0=AL.add, op1=AL.max)

                    if DEBUG and ci == 0 and b == 0:
                        nc.sync.dma_start(out=dbg_o1[:, 0, :], in_=o1r)
                        nc.sync.dma_start(out=dbg_o1[:, 1, :], in_=o1i)
                    t1 = CH.tile([128, 1024], F16, tag="t1")
                    t2 = CH.tile([128, 1024], F16, tag="t2")
                    t3r = CH.tile([128, 1024], F16, tag="t3r")
                    t3i = CH.tile([128, 1024], F16, tag="t3i")
                    ctb = CH.tile([128, 1024], F16, tag="ctb")

                    def layer(oa, ob, dcol0, dcol1, bcol0, bcol1, blam0, blam1,
                              extract_oi, tag):
                        """one fourierGC layer: returns (o_new_r, o_new_i?)"""
                        # t3r = oa*d0 - ob*d1
                        nc.vector.tensor_scalar_mul(t1[:, 0:cw], oa[:, 0:cw],
                                                    bc[:, dcol0:dcol0 + 1])
                        nc.vector.tensor_scalar_mul(t2[:, 0:cw], ob[:, 0:cw],
                                                    bc[:, dcol1:dcol1 + 1])
                        nc.vector.tensor_sub(t3r[:, 0:cw], t1[:, 0:cw], t2[:, 0:cw])
                        onr = CH.tile([128, 1024], F16, tag=tag + "r")
                        nc.scalar.activation(onr[:, 0:cw], t3r[:, 0:cw], AF.Relu,
                                             bias=bc[:, bcol0:bcol0 + 1])
                        nc.vector.tensor_scalar(out=ctb[:, 0:cw], in0=t3r[:, 0:cw],
                                                scalar1=bc[:, blam0:blam0 + 1],
                                                scalar2=0.0,
                                                op0=AL.add, op1=AL.max)
                        nc.vector.tensor_add(zacc_r[:, b, c0:c0 + cw],
                                             zacc_r[:, b, c0:c0 + cw],
                                             ctb[:, 0:cw])
                        # t3i = ob*d0 + onr*d1
                        nc.vector.tensor_scalar_mul(t1[:, 0:cw], ob[:, 0:cw],
                                                    bc[:, dcol0:dcol0 + 1])
                        nc.vector.tensor_scalar_mul(t2[:, 0:cw], onr[:, 0:cw],
                                                    bc[:, dcol1:dcol1 + 1])
                        nc.vector.tensor_add(t3i[:, 0:cw], t1[:, 0:cw], t2[:, 0:cw])
                        oni = None
                        if extract_oi:
                            oni = CH.tile([128, 1024], F16, tag=tag + "i")
                            nc.scalar.activation(oni[:, 0:cw], t3i[:, 0:cw],
                                                 AF.Relu,
                                                 bias=bc[:, bcol1:bcol1 + 1])
                        nc.vector.tensor_scalar(out=ctb[:, 0:cw], in0=t3i[:, 0:cw],
                                                scalar1=bc[:, blam1:blam1 + 1],
                                                scalar2=0.0,
                                                op0=AL.add, op1=AL.max)
                        nc.vector.tensor_add(zacc_i[:, b, c0:c0 + cw],
                                             zacc_i[:, b, c0:c0 + cw],
                                             ctb[:, 0:cw])
                        return onr, oni

                    o2r, o2i = layer(o1r, o1i, 0, 1, 4, 5, 8, 9, True, "o2")
                    layer(o2r, o2i, 2, 3, 6, 7, 10, 11, False, "o3")

                    nc.sync.dma_start_transpose(out=zt_r[:, 0:nt, b, :],
                                                in_=zacc_r[:, b, c0:c0 + cw])
                    nc.sync.dma_start_transpose(out=zt_i[:, 0:nt, b, :],
                                                in_=zacc_i[:, b, c0:c0 + cw])

                for t in range(nt):
                    tau = ci * 8 + t
                    for ri, ztt in ((0, zt_r), (1, zt_i)):
                        nc.tensor.matmul(
                            h0ps[:, :],
                            a16[:, tau * 16 + ri * 8:tau * 16 + ri * 8 + 8],
                            ztt[:, t, :, :],
                            start=(acc_i == 0), stop=(acc_i == nacc - 1))
                        acc_i += 1

            # ---- h0 completion + FC head ----
            v_sb = CH.tile([8, NB * 2], F32, tag="vsb")
            nc.vector.tensor_copy(v_sb, vps[:, :])
            h1in = CH.tile([8, NB, 128], F32, tag="h1in")
            tmp8 = CH.tile([8, 128], F32, tag="tmp8")
            for b in range(NB):
                nc.vector.tensor_scalar_mul(tmp8, emb8, v_sb[:, b:b + 1])
                nc.vector.tensor_add(h1in[:, b, :], tmp8,
                                     h0ps[:, b * 128:(b + 1) * 128])
            pst = PSM.tile([128, 2, 128], F32, tag="sm")
            hv = CH.tile([128, NB, 8], F32, tag="hv")
            for b in range(NB):
                nc.tensor.transpose(pst[:, b, 0:8], h1in[:, b, :],
                                    cf[0:8, O_ID:O_ID + 8])
                nc.vector.tensor_copy(hv[:, b, :], pst[:, b, 0:8])
            psf1 = PSM.tile([128, 2, 128], F32, tag="sm")
            for j in range(EMB):
                nc.tensor.matmul(psf1[0:64, 0, 0:NB],
                                 cf[:, O_W1 + j * 64:O_W1 + (j + 1) * 64],
                                 hv[:, :, j], start=(j == 0), stop=(j == EMB - 1))
            h1 = CH.tile([64, NB], F32, tag="h1")
            nc.scalar.activation(h1, psf1[0:64, 0, 0:NB], AF.Lrelu,
                                 bias=fcb[0:64, 0:1], scale=1.0, alpha=0.01)
            psf2 = PSM.tile([128, 2, 128], F32, tag="sm")
            h2 = CH.tile([128, 2, NB], F32, tag="h2")
            for h in range(2):
                nc.tensor.matmul(psf2[:, h, 0:NB], w2t[:, h * 128:(h + 1) * 128],
                                 h1, start=True, stop=True)
                nc.scalar.activation(h2[:, h, :], psf2[:, h, 0:NB], AF.Lrelu,
                                     bias=fcb[:, 1 + h:2 + h], scale=1.0,
                                     alpha=0.01)
            psf3 = PSM.tile([128, 2, 128], F32, tag="sm")
            for h in range(2):
                nc.tensor.matmul(psf3[0:96, 0, 0:NB],
                                 w3t[:, h * 96:(h + 1) * 96],
                                 h2[:, h, :], start=(h == 0), stop=(h == 1))
            if DEBUG:
                nc.sync.dma_start(out=dbg_zr[:, :, :], in_=zacc_r)
                nc.sync.dma_start(out=dbg_zi[:, :, :], in_=zacc_i)
                h0_sb = CH.tile([8, NB * 128], F32, tag="h0sb")
                nc.vector.tensor_copy(h0_sb, h0ps[:, :])
                nc.sync.dma_start(out=dbg_h0[:, :], in_=h0_sb)
                nc.sync.dma_start(out=dbg_v[:, :], in_=v_sb)
                nc.sync.dma_start(out=dbg_hv[:, :, :], in_=hv)
            out_sb = CH.tile([96, NB], F32, tag="outsb")
            nc.vector.tensor_scalar_add(out_sb, psf3[0:96, 0, 0:NB],
                                        fcb[0:96, 3:4])
            nc.sync.dma_start(out=out_d[:, :], in_=out_sb)

    nc.finalize()
    return nc


_NC_CACHE = None


def kernel(**inputs) -> np.ndarray:
    global _NC_CACHE
    if _NC_CACHE is None:
        _NC_CACHE = _build_nc()
    nc = _NC_CACHE

    consts = _host_constants(inputs)
    x = np.asarray(inputs["x"], np.float32)                   # [16, 512, 32]
    # xt[b] = x[b].T.flatten(); x2 = xt.reshape(128, 128)
    x2 = np.ascontiguousarray(x.transpose(0, 2, 1)).reshape(B, 128, 128)

    in_maps = []
    for c in range(NCORES):
        m = {"x2": x2[c * NB:(c + 1) * NB]}
        m.update(consts)
        in_maps.append(m)
    res = run_bass_kernel_spmd(nc, in_maps, core_ids=list(range(NCORES)))
    out = np.concatenate([r["out"].T for r in res.results], axis=0)
    return out.astype(np.float32)


# revision 10
# speedup vs baseline: 6.0926x; 1.2492x over previous
"""Trainium2 Bass kernel for nn_FGN_58007828300291 (Fourier Graph Network).

Math restructure (validated against the reference in fp64 to ~5e-7 rel):
  - tok = outer(xt, emb) is rank-1 and rfft is linear, so the big
    [B, 16384, 128] rfft collapses to a length-16384 rfft of xt per batch:
    Z[b,f,e] = X[b,f] * emb[e].
  - softshrink is always applied to relu outputs here, so
    softshrink(relu(v)) == relu(v - lambda); the chain is relu/fused ops only.
  - Only out[:, 0, :] survives downstream, so irfft + emb10 collapse into two
    precomputed [8, 8193] matrices (Ar, Ai):
       h0[b,e,j] = sum_f Ar[j,f] Sr[b,f,e] + Ai[j,f] Si[b,f,e]
  - S = z + Z splits into the small chain part z (fp16 through the
    contraction) and the rank-1 part Z, folded via v = Ar@Xr + Ai@Xi:
       h0 += v[j] * emb[e]
  - forward rfft(16384) = two 128x128 DFT matmul stages + twiddle; stage 2
    emits both X[k1,k2] (columns for the v matvec) and X^T (rows for the
    L1 outer-product matmuls).

Sharding: pure data parallel, B=16 over 8 cores (2 batches/core), params
replicated; host precomputes the small weight-derived constant matrices.
"""

import math

import ml_dtypes
import numpy as np

import concourse.bass as bass
import concourse.bacc as bacc
import concourse.tile as tile
from concourse import mybir
from concourse.bass_utils import run_bass_kernel_spmd

F16 = mybir.dt.float16
F32 = mybir.dt.float32
BF16 = mybir.dt.bfloat16
AL = mybir.AluOpType
AF = mybir.ActivationFunctionType

B, L, N = 16, 512, 32
E, EMB = 128, 8
NF = N * L              # 16384
F = NF // 2 + 1         # 8193
K2 = 65                 # f = 128*k2 + k1
FP = 128 * K2           # 8320 padded freq count (natural f order)
LAM = 0.01
NB = 2                  # batches per core
NCORES = 8

# cf (f32 [128, *]) column offsets
O_F1C, O_F1S = 0, 128
O_TWC, O_TWS = 256, 384
O_F2C, O_F2S, O_F2SN = 512, 577, 642
O_W1 = 707              # + j*64 + c   (512)
O_ID = 1219             # identity 128
CF_COLS = 1347

CHUNKS = [(i * 1024, 1024) for i in range(8)] + [(8192, 128)]
NTAU = FP // 128        # 65

# bc columns: 0-3 d10,d11,d20,d21; 4-7 b10,b11,b20,b21;
# 8-11 b10-lam..b21-lam; 12-13 b00-lam,b01-lam; 14-15 b00,b01
BC_COLS = 16


def _host_constants(inputs):
    emb = np.asarray(inputs["emb"], np.float64)[0]
    dg = lambda w: np.diagonal(np.asarray(w, np.float64), axis1=-2, axis2=-1)
    d00, d01 = dg(inputs["w0"][0]), dg(inputs["w0"][1])
    d10, d11 = dg(inputs["w1"][0]), dg(inputs["w1"][1])
    d20, d21 = dg(inputs["w2"][0]), dg(inputs["w2"][1])
    b00, b01 = np.asarray(inputs["b0"], np.float64)
    b10, b11 = np.asarray(inputs["b1"], np.float64)
    b20, b21 = np.asarray(inputs["b2"], np.float64)

    # fused inverse-DFT + emb10 matrices [8, F]
    t = np.arange(L)[:, None].astype(np.float64)
    f = np.arange(F)[None, :].astype(np.float64)
    c = np.full(F, 2.0); c[0] = 1.0; c[-1] = 1.0
    ang = 2.0 * np.pi * t * f / NF
    Cr = c * np.cos(ang) / math.sqrt(NF)
    Ci = -c * np.sin(ang) / math.sqrt(NF)
    emb10 = np.asarray(inputs["emb10"], np.float64)
    Ar = emb10.T @ Cr
    Ai = emb10.T @ Ci

    # natural-f padding to 8320 (zero A beyond 8192 kills the padded freqs)
    Apad = np.zeros((2, 8, FP))
    Apad[0][:, :F] = Ar
    Apad[1][:, :F] = Ai

    # a16 [128, NTAU*16]: tile tau covers f in [128*tau, 128*tau+128)
    a16 = np.zeros((128, NTAU * 16), np.float16)
    for tau in range(NTAU):
        for ri in range(2):
            a16[:, tau * 16 + ri * 8:tau * 16 + ri * 8 + 8] = \
                Apad[ri][:, tau * 128:(tau + 1) * 128].T.astype(np.float16)

    # cf [128, CF_COLS] f32
    cf = np.zeros((128, CF_COLS), np.float32)
    k = np.arange(128).astype(np.float64)
    kk = np.outer(k, k)
    cf[:, O_F1C:O_F1C + 128] = np.cos(2 * np.pi * kk / 128)
    cf[:, O_F1S:O_F1S + 128] = -np.sin(2 * np.pi * kk / 128)
    cf[:, O_TWC:O_TWC + 128] = np.cos(2 * np.pi * kk / NF)
    cf[:, O_TWS:O_TWS + 128] = -np.sin(2 * np.pi * kk / NF)
    kk2 = np.outer(k, np.arange(K2).astype(np.float64))
    cf[:, O_F2C:O_F2C + K2] = np.cos(2 * np.pi * kk2 / 128) / math.sqrt(NF)
    cf[:, O_F2S:O_F2S + K2] = -np.sin(2 * np.pi * kk2 / 128) / math.sqrt(NF)
    cf[:, O_F2SN:O_F2SN + K2] = np.sin(2 * np.pi * kk2 / 128) / math.sqrt(NF)
    fc1_w = np.asarray(inputs["fc1_w"], np.float64)
    w1r = fc1_w.reshape(64, 128, 8)
    for j in range(EMB):
        cf[:, O_W1 + j * 64:O_W1 + (j + 1) * 64] = w1r[:, :, j].T
    cf[:, O_ID:O_ID + 128] = np.eye(128)

    # c3 bf16 [2, 256]: L1 lhsT rows (for Xr, Xi) for o1r | o1i
    u00, u01 = emb * d00, emb * d01
    c3 = np.zeros((2, 256), np.float32)
    c3[0, :128], c3[1, :128] = u00, -u01
    c3[0, 128:], c3[1, 128:] = u01, u00

    bc = np.stack([d10, d11, d20, d21,
                   b10, b11, b20, b21,
                   b10 - LAM, b11 - LAM, b20 - LAM, b21 - LAM,
                   b00 - LAM, b01 - LAM, b00, b01],
                  axis=1).astype(np.float32)

    w2t = np.asarray(inputs["fc2_w"], np.float32).T.copy()
    fc3_w = np.asarray(inputs["fc3_w"], np.float64)
    w3t = np.zeros((128, 192), np.float32)
    w3t[:, 0:96] = fc3_w[:, 0:128].T
    w3t[:, 96:192] = fc3_w[:, 128:256].T
    fcb = np.zeros((128, 4), np.float32)
    fcb[0:64, 0] = np.asarray(inputs["fc1_b"], np.float32)
    fcb[:, 1] = np.asarray(inputs["fc2_b"], np.float32)[0:128]
    fcb[:, 2] = np.asarray(inputs["fc2_b"], np.float32)[128:256]
    fcb[0:96, 3] = np.asarray(inputs["fc3_b"], np.float32)
    embr = np.asarray(inputs["emb"], np.float32)

    return {
        "cf": cf, "c3": c3.astype(ml_dtypes.bfloat16), "a16": a16, "bc16": bc,
        "w2t": w2t, "w3t": w3t, "fcb": fcb, "embr": embr,
    }


DEBUG = False


def _build_nc():
    nc = bacc.Bacc(None)
    x2_d = nc.dram_tensor("x2", [NB, 128, 128], F32, kind="ExternalInput")
    cf_d = nc.dram_tensor("cf", [128, CF_COLS], F32, kind="ExternalInput")
    c3_d = nc.dram_tensor("c3", [2, 256], BF16, kind="ExternalInput")
    a16_d = nc.dram_tensor("a16", [128, NTAU * 16], F16, kind="ExternalInput")
    bc16_d = nc.dram_tensor("bc16", [128, BC_COLS], F32, kind="ExternalInput")
    w2t_d = nc.dram_tensor("w2t", [64, 256], F32, kind="ExternalInput")
    w3t_d = nc.dram_tensor("w3t", [128, 192], F32, kind="ExternalInput")
    fcb_d = nc.dram_tensor("fcb", [128, 4], F32, kind="ExternalInput")
    embr_d = nc.dram_tensor("embr", [1, 128], F32, kind="ExternalInput")
    out_d = nc.dram_tensor("out", [96, NB], F32, kind="ExternalOutput")
    if DEBUG:
        dbg_x = nc.dram_tensor("dbg_x", [128, K2, 2, NB], F16, kind="ExternalOutput")
        dbg_xs = nc.dram_tensor("dbg_xs", [2, NB, FP], BF16, kind="ExternalOutput")
        dbg_zr = nc.dram_tensor("dbg_zr", [128, NB, FP], F16, kind="ExternalOutput")
        dbg_zi = nc.dram_tensor("dbg_zi", [128, NB, FP], F16, kind="ExternalOutput")
        dbg_h0 = nc.dram_tensor("dbg_h0", [8, NB * 128], F32, kind="ExternalOutput")
        dbg_v = nc.dram_tensor("dbg_v", [8, NB * 2], F32, kind="ExternalOutput")
        dbg_hv = nc.dram_tensor("dbg_hv", [128, NB, 8], F32, kind="ExternalOutput")
        dbg_o1 = nc.dram_tensor("dbg_o1", [128, 2, 1024], F16, kind="ExternalOutput")

    with tile.TileContext(nc) as tc:
        with (
            tc.tile_pool(name="consts", bufs=1) as C,
            tc.tile_pool(name="chain", bufs=3) as CH,
            tc.tile_pool(name="zacc", bufs=1) as Z,
            tc.tile_pool(name="zt", bufs=2) as ZT,
            tc.tile_pool(name="psA", bufs=1, space="PSUM") as PSA,
            tc.tile_pool(name="psacc", bufs=1, space="PSUM") as PACC,
            tc.tile_pool(name="pssm", bufs=2, space="PSUM") as PSM,
        ):
            # ---- constants ----
            cf = C.tile([128, CF_COLS], F32)
            nc.gpsimd.dma_start(out=cf, in_=cf_d[:, :])
            c3 = C.tile([2, 256], BF16)
            nc.gpsimd.dma_start(out=c3, in_=c3_d[:, :])
            a16 = C.tile([128, NTAU * 16], F16)
            nc.gpsimd.dma_start(out=a16, in_=a16_d[:, :])
            bc = C.tile([128, BC_COLS], F32)
            nc.gpsimd.dma_start(out=bc, in_=bc16_d[:, :])
            w2t = C.tile([64, 256], F32)
            nc.gpsimd.dma_start(out=w2t, in_=w2t_d[:, :])
            w3t = C.tile([128, 192], F32)
            nc.gpsimd.dma_start(out=w3t, in_=w3t_d[:, :])
            fcb = C.tile([128, 4], F32)
            nc.gpsimd.dma_start(out=fcb, in_=fcb_d[:, :])
            emb8 = C.tile([8, 128], F32)
            embr_ap = embr_d[:, :]
            nc.gpsimd.dma_start(
                out=emb8,
                in_=bass.AP(tensor=embr_ap.tensor, offset=embr_ap.offset,
                            ap=[[0, 8], [1, 128]]))

            xstage = C.tile([2, NB, FP], BF16)
            xc16 = C.tile([128, K2, 2, NB], F16)    # X columns [k1, k2, ri, b]
            xbt = C.tile([65, 2, NB, 128], BF16)    # X^T rows  [k2, ri, b, k1]
            zacc_r = Z.tile([128, NB, FP], F16)
            zacc_i = Z.tile([128, NB, FP], F16)

            h0ps = PACC.tile([8, NB * 128], F32, tag="h0")
            vps = PACC.tile([8, NB], F32, tag="v")

            # ---- forward FFT per batch ----
            for b in range(NB):
                x2 = CH.tile([128, 128], F32, tag="x2")
                nc.sync.dma_start(out=x2, in_=x2_d[b, :, :])
                ps1 = PSM.tile([128, 2, 128], F32, tag="sm")
                nc.tensor.matmul(ps1[:, 0, :], x2, cf[:, O_F1C:O_F1C + 128],
                                 start=True, stop=True)
                nc.tensor.matmul(ps1[:, 1, :], x2, cf[:, O_F1S:O_F1S + 128],
                                 start=True, stop=True)
                ap_r = CH.tile([128, 128], F32, tag="apr")
                ap_i = CH.tile([128, 128], F32, tag="api")
                tw1 = CH.tile([128, 128], F32, tag="tw1")
                tw2 = CH.tile([128, 128], F32, tag="tw2")
                nc.vector.tensor_mul(tw1, ps1[:, 0, :], cf[:, O_TWC:O_TWC + 128])
                nc.vector.tensor_mul(tw2, ps1[:, 1, :], cf[:, O_TWS:O_TWS + 128])
                nc.vector.tensor_sub(ap_r, tw1, tw2)
                nc.vector.tensor_mul(tw1, ps1[:, 0, :], cf[:, O_TWS:O_TWS + 128])
                nc.vector.tensor_mul(tw2, ps1[:, 1, :], cf[:, O_TWC:O_TWC + 128])
                nc.vector.tensor_add(ap_i, tw1, tw2)
                # X [k1, k2] (v-matvec columns) and X^T [k2, k1] (L1 rows)
                ps2 = PSM.tile([128, 2, 128], F32, tag="sm")
                nc.tensor.matmul(ps2[:, 0, 0:K2], ap_r, cf[:, O_F2C:O_F2C + K2],
                                 start=True, stop=False)
                nc.tensor.matmul(ps2[:, 0, 0:K2], ap_i, cf[:, O_F2SN:O_F2SN + K2],
                                 start=False, stop=True)
                nc.tensor.matmul(ps2[:, 1, 0:K2], ap_r, cf[:, O_F2S:O_F2S + K2],
                                 start=True, stop=False)
                nc.tensor.matmul(ps2[:, 1, 0:K2], ap_i, cf[:, O_F2C:O_F2C + K2],
                                 start=False, stop=True)
                ps3 = PSM.tile([128, 2, 128], F32, tag="sm")
                nc.tensor.matmul(ps3[0:K2, 0, :], cf[:, O_F2C:O_F2C + K2], ap_r,
                                 start=True, stop=False)
                nc.tensor.matmul(ps3[0:K2, 0, :], cf[:, O_F2SN:O_F2SN + K2], ap_i,
                                 start=False, stop=True)
                nc.tensor.matmul(ps3[0:K2, 1, :], cf[:, O_F2S:O_F2S + K2], ap_r,
                                 start=True, stop=False)
                nc.tensor.matmul(ps3[0:K2, 1, :], cf[:, O_F2C:O_F2C + K2], ap_i,
                                 start=False, stop=True)
                for ri in range(2):
                    nc.vector.tensor_copy(xc16[:, :, ri, b], ps2[:, ri, 0:K2])
                    nc.scalar.copy(xbt[:, ri, b, :], ps3[0:K2, ri, :])
                    nc.sync.dma_start(out=xstage[ri:ri + 1, b, :],
                                      in_=xbt[:, ri, b, :])

            # ---- chain + transpose + contraction ----
            nacc = 2 * 2 * NTAU
            acc_i = 0
            acc_v = 0
            for ci, (c0, cw) in enumerate(CHUNKS):
                nt = cw // 128
                zt_r = ZT.tile([128, 8, NB, 128], F16, tag="ztr")
                zt_i = ZT.tile([128, 8, NB, 128], F16, tag="zti")
                for b in range(NB):
                    psA = PSA.tile([128, 2, 1024], F32, tag="psA")
                    for h0_ in range(0, cw, 512):
                        hw_ = min(512, cw - h0_)
                        nc.tensor.matmul(psA[:, 0, h0_:h0_ + hw_], c3[:, 0:128],
                                         xstage[:, b, c0 + h0_:c0 + h0_ + hw_],
                                         start=True, stop=True)
                        nc.tensor.matmul(psA[:, 1, h0_:h0_ + hw_], c3[:, 128:256],
                                         xstage[:, b, c0 + h0_:c0 + h0_ + hw_],
                                         start=True, stop=True)
                    o1r = CH.tile([128, 1024], F16, tag="o1r")
                    o1i = CH.tile([128, 1024], F16, tag="o1i")
                    nc.scalar.activation(o1r[:, 0:cw], psA[:, 0, 0:cw], AF.Relu,
                                         bias=bc[:, 14:15])
                    nc.scalar.activation(o1i[:, 0:cw], psA[:, 1, 0:cw], AF.Relu,
                                         bias=bc[:, 15:16])
                    nc.scalar.activation(zacc_r[:, b, c0:c0 + cw], psA[:, 0, 0:cw],
                                         AF.Relu, bias=bc[:, 12:13])
                    nc.scalar.activation(zacc_i[:, b, c0:c0 + cw], psA[:, 1, 0:cw],
                                         AF.Relu, bias=bc[:, 13:14])
                    if DEBUG and ci == 0 and b == 0:
                        nc.sync.dma_start(out=dbg_o1[:, 0, :], in_=o1r)
                        nc.sync.dma_start(out=dbg_o1[:, 1, :], in_=o1i)

                    t1 = CH.tile([128, 1024], F16, tag="t1")
                    t2 = CH.tile([128, 1024], F16, tag="t2")
                    t3r = CH.tile([128, 1024], F16, tag="t3r")
                    t3i = CH.tile([128, 1024], F16, tag="t3i")
                    ctb = CH.tile([128, 1024], F16, tag="ctb")

                    def layer(oa, ob, dcol0, dcol1, bcol0, bcol1, blam0, blam1,
                              extract_oi, tag):
                        nc.vector.tensor_scalar_mul(t1[:, 0:cw], oa[:, 0:cw],
                                                    bc[:, dcol0:dcol0 + 1])
                        nc.vector.tensor_scalar_mul(t2[:, 0:cw], ob[:, 0:cw],
                                                    bc[:, dcol1:dcol1 + 1])
                        nc.vector.tensor_sub(t3r[:, 0:cw], t1[:, 0:cw], t2[:, 0:cw])
                        onr = CH.tile([128, 1024], F16, tag=tag + "r")
                        nc.scalar.activation(onr[:, 0:cw], t3r[:, 0:cw], AF.Relu,
                                             bias=bc[:, bcol0:bcol0 + 1])
                        nc.vector.tensor_scalar(out=ctb[:, 0:cw], in0=t3r[:, 0:cw],
                                                scalar1=bc[:, blam0:blam0 + 1],
                                                scalar2=0.0,
                                                op0=AL.add, op1=AL.max)
                        nc.vector.tensor_add(zacc_r[:, b, c0:c0 + cw],
                                             zacc_r[:, b, c0:c0 + cw],
                                             ctb[:, 0:cw])
                        nc.vector.tensor_scalar_mul(t1[:, 0:cw], ob[:, 0:cw],
                                                    bc[:, dcol0:dcol0 + 1])
                        nc.vector.tensor_scalar_mul(t2[:, 0:cw], onr[:, 0:cw],
                                                    bc[:, dcol1:dcol1 + 1])
                        nc.vector.tensor_add(t3i[:, 0:cw], t1[:, 0:cw], t2[:, 0:cw])
                        oni = None
                        if extract_oi:
                            oni = CH.tile([128, 1024], F16, tag=tag + "i")
                            nc.scalar.activation(oni[:, 0:cw], t3i[:, 0:cw],
                                                 AF.Relu,
                                                 bias=bc[:, bcol1:bcol1 + 1])
                        nc.vector.tensor_scalar(out=ctb[:, 0:cw], in0=t3i[:, 0:cw],
                                                scalar1=bc[:, blam1:blam1 + 1],
                                                scalar2=0.0,
                                                op0=AL.add, op1=AL.max)
                        nc.vector.tensor_add(zacc_i[:, b, c0:c0 + cw],
                                             zacc_i[:, b, c0:c0 + cw],
                                             ctb[:, 0:cw])
                        return onr, oni

                    o2r, o2i = layer(o1r, o1i, 0, 1, 4, 5, 8, 9, True, "o2")
                    layer(o2r, o2i, 2, 3, 6, 7, 10, 11, False, "o3")

                    nc.sync.dma_start_transpose(out=zt_r[:, 0:nt, b, :],
                                                in_=zacc_r[:, b, c0:c0 + cw])
                    nc.sync.dma_start_transpose(out=zt_i[:, 0:nt, b, :],
                                                in_=zacc_i[:, b, c0:c0 + cw])

                for t in range(nt):
                    tau = ci * 8 + t
                    for ri, ztt in ((0, zt_r), (1, zt_i)):
                        aslice = a16[:, tau * 16 + ri * 8:tau * 16 + ri * 8 + 8]
                        nc.tensor.matmul(
                            h0ps[:, :], aslice, ztt[:, t, :, :],
                            start=(acc_i == 0), stop=(acc_i == nacc - 1))
                        acc_i += 1
                        nc.tensor.matmul(
                            vps[:, :], aslice, xc16[:, tau, ri, :],
                            start=(acc_v == 0), stop=(acc_v == nacc - 1))
                        acc_v += 1

            # ---- h0 completion + FC head ----
            v_sb = CH.tile([8, NB], F32, tag="vsb")
            nc.vector.tensor_copy(v_sb, vps[:, :])
            h1in = CH.tile([8, NB, 128], F32, tag="h1in")
            tmp8 = CH.tile([8, 128], F32, tag="tmp8")
            for b in range(NB):
                nc.vector.tensor_scalar_mul(tmp8, emb8, v_sb[:, b:b + 1])
                nc.vector.tensor_add(h1in[:, b, :], tmp8,
                                     h0ps[:, b * 128:(b + 1) * 128])
            pst = PSM.tile([128, 2, 128], F32, tag="sm")
            hv = CH.tile([128, NB, 8], F32, tag="hv")
            for b in range(NB):
                nc.tensor.transpose(pst[:, b, 0:8], h1in[:, b, :],
                                    cf[0:8, O_ID:O_ID + 8])
                nc.vector.tensor_copy(hv[:, b, :], pst[:, b, 0:8])
            psf1 = PSM.tile([128, 2, 128], F32, tag="sm")
            for j in range(EMB):
                nc.tensor.matmul(psf1[0:64, 0, 0:NB],
                                 cf[:, O_W1 + j * 64:O_W1 + (j + 1) * 64],
                                 hv[:, :, j], start=(j == 0), stop=(j == EMB - 1))
            h1 = CH.tile([64, NB], F32, tag="h1")
            nc.scalar.activation(h1, psf1[0:64, 0, 0:NB], AF.Lrelu,
                                 bias=fcb[0:64, 0:1], scale=1.0, alpha=0.01)
            psf2 = PSM.tile([128, 2, 128], F32, tag="sm")
            h2 = CH.tile([128, 2, NB], F32, tag="h2")
            for h in range(2):
                nc.tensor.matmul(psf2[:, h, 0:NB], w2t[:, h * 128:(h + 1) * 128],
                                 h1, start=True, stop=True)
                nc.scalar.activation(h2[:, h, :], psf2[:, h, 0:NB], AF.Lrelu,
                                     bias=fcb[:, 1 + h:2 + h], scale=1.0,
                                     alpha=0.01)
            psf3 = PSM.tile([128, 2, 128], F32, tag="sm")
            for h in range(2):
                nc.tensor.matmul(psf3[0:96, 0, 0:NB],
                                 w3t[:, h * 96:(h + 1) * 96],
                                 h2[:, h, :], start=(h == 0), stop=(h == 1))
            out_sb = CH.tile([96, NB], F32, tag="outsb")
            nc.vector.tensor_scalar_add(out_sb, psf3[0:96, 0, 0:NB],
                                        fcb[0:96, 3:4])
            nc.sync.dma_start(out=out_d[:, :], in_=out_sb)

            if DEBUG:
                nc.sync.dma_start(out=dbg_x[:, :, :, :], in_=xc16)
                nc.sync.dma_start(out=dbg_xs[:, :, :], in_=xstage)
                nc.sync.dma_start(out=dbg_zr[:, :, :], in_=zacc_r)
                nc.sync.dma_start(out=dbg_zi[:, :, :], in_=zacc_i)
                h0_sb = CH.tile([8, NB * 128], F32, tag="h0sb")
                nc.vector.tensor_copy(h0_sb, h0ps[:, :])
                nc.sync.dma_start(out=dbg_h0[:, :], in_=h0_sb)
                v2_sb = CH.tile([8, NB * 2], F32, tag="v2sb")
                nc.vector.tensor_copy(v2_sb[:, 0:NB], vps[:, :])
                nc.vector.tensor_copy(v2_sb[:, NB:], vps[:, :])
                nc.sync.dma_start(out=dbg_v[:, :], in_=v2_sb)
                nc.sync.dma_start(out=dbg_hv[:, :, :], in_=hv)

    nc.finalize()
    return nc


_NC_CACHE = None


def kernel(**inputs) -> np.ndarray:
    global _NC_CACHE
    if _NC_CACHE is None:
        _NC_CACHE = _build_nc()
    nc = _NC_CACHE

    consts = _host_constants(inputs)
    x = np.asarray(inputs["x"], np.float32)
    x2 = np.ascontiguousarray(x.transpose(0, 2, 1)).reshape(B, 128, 128)

    in_maps = []
    for c in range(NCORES):
        m = {"x2": x2[c * NB:(c + 1) * NB]}
        m.update(consts)
        in_maps.append(m)
    res = run_bass_kernel_spmd(nc, in_maps, core_ids=list(range(NCORES)))
    out = np.concatenate([r["out"].T for r in res.results], axis=0)
    return out.astype(np.float32)
